# revision 16
# baseline (speedup 1.0000x reference)
"""Trainium2 Bass kernel for nn_AttentionMechanism (tanh-MLP attention).

Math (per batch b):
  q[:, b]   = W_h_w @ h_t[b] + W_h_b + W_b                  (host, tiny)
  U[beta,s,b] = sum_c W_w[beta,c] V[c,s,b]                   (PE)
  T = tanh(U + q)     (q folded in as the ACT per-partition bias)
  E[s,b]    = sum_beta bw[beta] T[beta,s,b]                  (PE, output replicated over partitions)
  w = exp(E)          (no max-subtraction needed: |E| <= ||bw||_1 ~ 8)
  P[c,b]    = sum_s w[s,b] V[c,s,b]                          (DVE affine_mul_reduce)
  SE[b]     = sum_s w[s,b]                                   (DVE tensor_scalar accum)
  C[b,0,c]  = sum_cores P / sum_cores SE                     (host, tiny)

Sharding: 2D - 4-way over positions (hp quarters) x 2-way over batch
halves.  Each core gets s=1024 positions x 32 batches (32MB of V);
softmax combined on host over the 4 position-shards of each batch half.
The s=1024 per (core, batch) makes every ACT instruction FD>=1024,
amortizing the per-instruction overhead that bounded the 1D version.

Host pre-lays V out per-core as [c, b, s] bf16 (the sharding-prep copy),
so the device DMA reads contiguous runs at full HBM bandwidth, every
matmul rhs is s-contiguous (full PE rate), and the DVE P stage is a
single fused multiply-accumulate per (c-chunk, batch).
"""

import sys
from contextlib import ExitStack

import numpy as np

if "/opt/trn_rl_repo" not in sys.path:
    sys.path.insert(0, "/opt/trn_rl_repo")

import ml_dtypes

BF16 = ml_dtypes.bfloat16

HP, WP, C_DIM, B = 64, 64, 256, 64
BETA, HIDDEN = 512, 512
NCORES = 8
N_HPQ = 4                      # position shards
N_BH = 2                       # batch shards
B_CORE = B // N_BH             # 32 batches per core
S_CORE = (HP // N_HPQ) * WP    # 1024 positions per core
B_OCT = 2                      # batches per DMA tile

_NC_CACHE = {}


def _build_nc(s_core=S_CORE):
    import concourse.bass as bass
    import concourse.bacc as bacc
    import concourse.tile as tile
    import concourse.mybir as mybir
    from concourse.mybir import dt

    AF = mybir.ActivationFunctionType
    ALU = mybir.AluOpType
    f32, bf16 = dt.float32, dt.bfloat16

    n_oct = B_CORE // B_OCT
    n_sh = s_core // 512           # matmul N=512 tiles per batch

    nc = bacc.Bacc("TRN2", target_bir_lowering=False, debug=False,
                   num_devices=NCORES)

    v_d = nc.dram_tensor("v", [C_DIM, B_CORE, s_core], bf16,
                         kind="ExternalInput")
    wt_d = nc.dram_tensor("wt", [128, 2 * BETA], bf16, kind="ExternalInput")
    qs_d = nc.dram_tensor("qs", [128, 4 * B_CORE], f32, kind="ExternalInput")
    bwr_d = nc.dram_tensor("bwr", [128, BETA], bf16, kind="ExternalInput")
    p_d = nc.dram_tensor("p_out", [2, 128, B_CORE], f32, kind="ExternalOutput")
    se_d = nc.dram_tensor("se_out", [1, B_CORE], f32, kind="ExternalOutput")

    with tile.TileContext(nc) as tc, ExitStack() as ctx:
        cpool = ctx.enter_context(tc.tile_pool(name="const", bufs=1))
        vpool = ctx.enter_context(tc.tile_pool(name="vp", bufs=1))
        tpool = ctx.enter_context(tc.tile_pool(name="tp", bufs=5))
        wpool = ctx.enter_context(tc.tile_pool(name="wp", bufs=2))
        ppool = ctx.enter_context(tc.tile_pool(name="pp", bufs=2))
        apool = ctx.enter_context(tc.tile_pool(name="ap", bufs=1))
        psum = ctx.enter_context(tc.tile_pool(name="ps", bufs=4, space="PSUM"))

        # ---- constants ----
        wt_sb = cpool.tile([128, 2 * BETA], bf16, tag="wt")
        nc.sync.dma_start(wt_sb, wt_d[:])
        qs_sb = cpool.tile([128, 4 * B_CORE], f32, tag="qs")
        nc.sync.dma_start(qs_sb, qs_d[:])
        bwr_sb = cpool.tile([128, BETA], bf16, tag="bwr")
        nc.sync.dma_start(bwr_sb, bwr_d[:])

        # ---- V tiles resident; first pair as single-b tiles, DMAs spread
        # across both HWDGE queues (sync+scalar) so compute starts early ----
        vb = [[None, None] for _ in range(B_CORE)]
        for b in range(B_OCT):
            for k in range(2):
                eng = nc.sync if (b + k) % 2 == 0 else nc.scalar
                t = vpool.tile([128, s_core], bf16, tag=f"vs{k}b{b}",
                               name=f"vs{k}b{b}")
                eng.dma_start(t, v_d[k * 128:(k + 1) * 128, b, :])
                vb[b][k] = t
        for o in range(1, n_oct):
            for k in range(2):
                eng = nc.sync if (o + k) % 2 == 0 else nc.scalar
                t = vpool.tile([128, B_OCT * s_core], bf16, tag=f"v{k}o{o}",
                               name=f"v{k}o{o}")
                eng.dma_start(
                    t, v_d[k * 128:(k + 1) * 128, o * B_OCT:(o + 1) * B_OCT, :])
                view = t.rearrange("p (b s) -> p b s", s=s_core)
                for h in range(B_OCT):
                    vb[o * B_OCT + h][k] = view[:, h, :]

        # ---- output accumulators ----
        p_fin = [apool.tile([128, B_CORE], f32, tag=f"pfin{k}",
                            name=f"pfin{k}") for k in range(2)]
        se_fin = apool.tile([128, B_CORE], f32, tag="sefin")

        for b in range(B_CORE):
            t_tiles = []
            for m in range(4):
                u = psum.tile([128, s_core], f32, tag="acc", name="u")
                for kp in range(2):
                    for sh in range(n_sh):
                        nc.tensor.matmul(
                            u[:, sh * 512:(sh + 1) * 512],
                            wt_sb[:, kp * BETA + m * 128:
                                  kp * BETA + (m + 1) * 128],
                            vb[b][kp][:, sh * 512:(sh + 1) * 512],
                            start=(kp == 0), stop=(kp == 1))
                t_m = tpool.tile([128, s_core], bf16, tag="t", name="t_m")
                nc.scalar.activation(
                    t_m, u, AF.Tanh,
                    bias=qs_sb[:, m * B_CORE + b:m * B_CORE + b + 1])
                t_tiles.append(t_m)

            e_rep = psum.tile([128, s_core], f32, tag="acc", name="e_rep")
            for m in range(4):
                for sh in range(n_sh):
                    nc.tensor.matmul(
                        e_rep[:, sh * 512:(sh + 1) * 512],
                        bwr_sb[:, m * 128:(m + 1) * 128],
                        t_tiles[m][:, sh * 512:(sh + 1) * 512],
                        start=(m == 0), stop=(m == 3))
            w_rep = wpool.tile([128, s_core], bf16, tag="w", name="w_rep")
            nc.scalar.activation(w_rep, e_rep, AF.Exp)

            for k in range(2):
                prod = ppool.tile([128, s_core], bf16, tag="prod",
                                  name="prod")
                nc.vector.affine_mul_reduce(
                    out=prod, accum_out=p_fin[k][:, b:b + 1],
                    in0=vb[b][k], in1=w_rep,
                    scale=1.0, bias=0.0)
            sescr = ppool.tile([128, s_core], bf16, tag="sescr",
                               name="sescr")
            nc.vector.tensor_scalar(
                sescr, w_rep, 1.0, None, op0=ALU.mult, op1=ALU.add,
                accum_out=se_fin[:, b:b + 1])

        for k in range(2):
            nc.sync.dma_start(p_d[k], p_fin[k])
        nc.sync.dma_start(se_d[:], se_fin[0:1, :])

    nc.compile()
    return nc


def _get_nc(s_core=S_CORE):
    if s_core not in _NC_CACHE:
        _NC_CACHE[s_core] = _build_nc(s_core)
    return _NC_CACHE[s_core]


def _host_smalls(h_t, W_h_w, W_h_b, W_w, W_b, beta_w):
    q = h_t[:, 0, :].astype(np.float64) @ W_h_w.T.astype(np.float64) \
        + W_h_b + W_b                                  # [b, beta]
    # per batch-half: qs[p, m*B_CORE+b] = q[bh*B_CORE+b, m*128+p]
    qs3 = q.T.reshape(4, 128, B).transpose(1, 0, 2)    # [128, 4, 64]
    qs_h = [np.ascontiguousarray(
        qs3[:, :, bh * B_CORE:(bh + 1) * B_CORE].reshape(128, 4 * B_CORE)
    ).astype(np.float32) for bh in range(N_BH)]
    wt = np.ascontiguousarray(
        W_w.T.reshape(2, 128, BETA).transpose(1, 0, 2).reshape(128, 2 * BETA)
    ).astype(BF16)
    bw = beta_w[0].astype(np.float32)
    bwr = np.ascontiguousarray(
        np.repeat(bw.reshape(4, 128).T[:, :, None], 128, axis=2).reshape(128, BETA)
    ).astype(BF16)
    return qs_h, wt, bwr


_PROFILE = False
_LAST_PERF = {}


def kernel(**inputs):
    from concourse.bass_utils import run_bass_kernel_spmd

    V = np.asarray(inputs["V"], dtype=np.float32)
    h_t = np.asarray(inputs["h_t"], dtype=np.float32)
    W_h_w = np.asarray(inputs["W_h_w"], dtype=np.float32)
    W_h_b = np.asarray(inputs["W_h_b"], dtype=np.float32)
    W_w = np.asarray(inputs["W_w"], dtype=np.float32)
    W_b = np.asarray(inputs["W_b"], dtype=np.float32)
    beta_w = np.asarray(inputs["beta_w"], dtype=np.float32)
    beta_b = np.asarray(inputs["beta_b"], dtype=np.float32)

    qs_h, wt, bwr = _host_smalls(h_t, W_h_w, W_h_b, W_w, W_b, beta_w)

    rows = HP // N_HPQ
    Vb = V.astype(BF16)
    in_maps = []
    core_meta = []
    for k in range(N_HPQ):
        Vq = Vb[k * rows:(k + 1) * rows].reshape(S_CORE, C_DIM, B)
        for bh in range(N_BH):
            # [s, c, b-half] -> [c, b, s] contiguous
            vk = np.ascontiguousarray(
                Vq[:, :, bh * B_CORE:(bh + 1) * B_CORE].transpose(1, 2, 0))
            in_maps.append({"v": vk, "wt": wt, "qs": qs_h[bh], "bwr": bwr})
            core_meta.append(bh)

    nc = _get_nc()
    res = run_bass_kernel_spmd(nc, in_maps, core_ids=list(range(NCORES)),
                               trace=_PROFILE)
    if _PROFILE:
        _LAST_PERF["exec_time_ns"] = res.exec_time_ns
        _LAST_PERF["trace"] = res.instructions_and_trace
    P = np.zeros((C_DIM, B), np.float64)
    SE = np.zeros((B,), np.float64)
    for bh, r in zip(core_meta, res.results):
        sl = slice(bh * B_CORE, (bh + 1) * B_CORE)
        P[:, sl] += r["p_out"].reshape(C_DIM, B_CORE)
        SE[sl] += r["se_out"][0]
    # softmax is shift-invariant so beta_b cancels; no max-sub needed (|E|<=~8)
    C = (P / SE).T.reshape(B, 1, C_DIM)
    return C.astype(np.float32)


# revision 17
# speedup vs baseline: 1.1420x; 1.1420x over previous
"""Trainium2 Bass kernel for nn_AttentionMechanism (tanh-MLP attention).

Math (per batch b):
  q[:, b]   = W_h_w @ h_t[b] + W_h_b + W_b                  (host, tiny)
  U[beta,s,b] = sum_c W_w[beta,c] V[c,s,b]                   (PE)
  T = tanh(U + q)     (q folded in as the ACT per-partition bias)
  E[s,b]    = sum_beta bw[beta] T[beta,s,b]                  (PE, output replicated over partitions)
  w = exp(E)          (no max-subtraction needed: |E| <= ||bw||_1 ~ 8)
  P[c,b]    = sum_s w[s,b] V[c,s,b]                          (DVE affine_mul_reduce)
  SE[b]     = sum_s w[s,b]                                   (DVE tensor_scalar accum)
  C[b,0,c]  = sum_cores P / sum_cores SE                     (host, tiny)

Sharding: 2D - 4-way over positions (hp quarters) x 2-way over batch
halves.  Each core gets s=1024 positions x 32 batches (32MB of V);
softmax combined on host over the 4 position-shards of each batch half.
The s=1024 per (core, batch) makes every ACT instruction FD>=1024,
amortizing the per-instruction overhead that bounded the 1D version.

Host pre-lays V out per-core as [c, b, s] bf16 (the sharding-prep copy),
so the device DMA reads contiguous runs at full HBM bandwidth, every
matmul rhs is s-contiguous (full PE rate), and the DVE P stage is a
single fused multiply-accumulate per (c-chunk, batch).
"""

import sys
from contextlib import ExitStack

import numpy as np

if "/opt/trn_rl_repo" not in sys.path:
    sys.path.insert(0, "/opt/trn_rl_repo")

import ml_dtypes

BF16 = ml_dtypes.bfloat16

HP, WP, C_DIM, B = 64, 64, 256, 64
BETA, HIDDEN = 512, 512
NCORES = 8
N_HPQ = 4                      # position shards
N_BH = 2                       # batch shards
B_CORE = B // N_BH             # 32 batches per core
S_CORE = (HP // N_HPQ) * WP    # 1024 positions per core
B_OCT = 2                      # batches per DMA tile

_NC_CACHE = {}


def _build_nc(s_core=S_CORE):
    import concourse.bass as bass
    import concourse.bacc as bacc
    import concourse.tile as tile
    import concourse.mybir as mybir
    from concourse.mybir import dt

    AF = mybir.ActivationFunctionType
    ALU = mybir.AluOpType
    f32, bf16 = dt.float32, dt.bfloat16

    n_oct = B_CORE // B_OCT
    n_sh = s_core // 512           # matmul N=512 tiles per batch

    nc = bacc.Bacc("TRN2", target_bir_lowering=False, debug=False,
                   num_devices=NCORES)

    v_d = nc.dram_tensor("v", [C_DIM, B_CORE, s_core], bf16,
                         kind="ExternalInput")
    wt_d = nc.dram_tensor("wt", [128, 2 * BETA], bf16, kind="ExternalInput")
    qs_d = nc.dram_tensor("qs", [128, 4 * B_CORE], f32, kind="ExternalInput")
    bwr_d = nc.dram_tensor("bwr", [128, BETA], bf16, kind="ExternalInput")
    p_d = nc.dram_tensor("p_out", [2, 128, B_CORE], f32, kind="ExternalOutput")
    se_d = nc.dram_tensor("se_out", [1, B_CORE], f32, kind="ExternalOutput")

    with tile.TileContext(nc) as tc, ExitStack() as ctx:
        cpool = ctx.enter_context(tc.tile_pool(name="const", bufs=1))
        vpool = ctx.enter_context(tc.tile_pool(name="vp", bufs=1))
        tpool = ctx.enter_context(tc.tile_pool(name="tp", bufs=5))
        wpool = ctx.enter_context(tc.tile_pool(name="wp", bufs=2))
        ppool = ctx.enter_context(tc.tile_pool(name="pp", bufs=2))
        apool = ctx.enter_context(tc.tile_pool(name="ap", bufs=1))
        psum = ctx.enter_context(tc.tile_pool(name="ps", bufs=4, space="PSUM"))

        # ---- constants ----
        wt_sb = cpool.tile([128, 2 * BETA], bf16, tag="wt")
        nc.sync.dma_start(wt_sb, wt_d[:])
        qs_sb = cpool.tile([128, 4 * B_CORE], f32, tag="qs")
        nc.sync.dma_start(qs_sb, qs_d[:])
        bwr_sb = cpool.tile([128, BETA], bf16, tag="bwr")
        nc.sync.dma_start(bwr_sb, bwr_d[:])

        # ---- V tiles: [c-chunk][b-pair] resident, DMA'd in order ----
        vv = [[None, None] for _ in range(n_oct)]
        for o in range(n_oct):
            for k in range(2):
                t = vpool.tile([128, B_OCT * s_core], bf16, tag=f"v{k}o{o}",
                               name=f"v{k}o{o}")
                nc.sync.dma_start(
                    t, v_d[k * 128:(k + 1) * 128, o * B_OCT:(o + 1) * B_OCT, :])
                vv[o][k] = t.rearrange("p (b s) -> p b s", s=s_core)

        # ---- output accumulators ----
        p_fin = [apool.tile([128, B_CORE], f32, tag=f"pfin{k}",
                            name=f"pfin{k}") for k in range(2)]
        se_fin = apool.tile([128, B_CORE], f32, tag="sefin")

        for b in range(B_CORE):
            o, h = divmod(b, B_OCT)
            t_tiles = []
            for m in range(4):
                u = psum.tile([128, s_core], f32, tag="acc", name="u")
                for kp in range(2):
                    for sh in range(n_sh):
                        nc.tensor.matmul(
                            u[:, sh * 512:(sh + 1) * 512],
                            wt_sb[:, kp * BETA + m * 128:
                                  kp * BETA + (m + 1) * 128],
                            vv[o][kp][:, h, sh * 512:(sh + 1) * 512],
                            start=(kp == 0), stop=(kp == 1))
                t_m = tpool.tile([128, s_core], bf16, tag="t", name="t_m")
                nc.scalar.activation(
                    t_m, u, AF.Tanh,
                    bias=qs_sb[:, m * B_CORE + b:m * B_CORE + b + 1])
                t_tiles.append(t_m)

            e_rep = psum.tile([128, s_core], f32, tag="acc", name="e_rep")
            for m in range(4):
                for sh in range(n_sh):
                    nc.tensor.matmul(
                        e_rep[:, sh * 512:(sh + 1) * 512],
                        bwr_sb[:, m * 128:(m + 1) * 128],
                        t_tiles[m][:, sh * 512:(sh + 1) * 512],
                        start=(m == 0), stop=(m == 3))
            w_rep = wpool.tile([128, s_core], bf16, tag="w", name="w_rep")
            nc.scalar.activation(w_rep, e_rep, AF.Exp)

            for k in range(2):
                prod = ppool.tile([128, s_core], bf16, tag="prod",
                                  name="prod")
                nc.vector.affine_mul_reduce(
                    out=prod, accum_out=p_fin[k][:, b:b + 1],
                    in0=vv[o][k][:, h, :], in1=w_rep,
                    scale=1.0, bias=0.0)
            sescr = ppool.tile([128, s_core], bf16, tag="sescr",
                               name="sescr")
            nc.vector.tensor_scalar(
                sescr, w_rep, 1.0, None, op0=ALU.mult, op1=ALU.add,
                accum_out=se_fin[:, b:b + 1])

        for k in range(2):
            nc.sync.dma_start(p_d[k], p_fin[k])
        nc.sync.dma_start(se_d[:], se_fin[0:1, :])

    nc.compile()
    return nc


def _get_nc(s_core=S_CORE):
    if s_core not in _NC_CACHE:
        _NC_CACHE[s_core] = _build_nc(s_core)
    return _NC_CACHE[s_core]


def _host_smalls(h_t, W_h_w, W_h_b, W_w, W_b, beta_w):
    q = h_t[:, 0, :].astype(np.float64) @ W_h_w.T.astype(np.float64) \
        + W_h_b + W_b                                  # [b, beta]
    # per batch-half: qs[p, m*B_CORE+b] = q[bh*B_CORE+b, m*128+p]
    qs3 = q.T.reshape(4, 128, B).transpose(1, 0, 2)    # [128, 4, 64]
    qs_h = [np.ascontiguousarray(
        qs3[:, :, bh * B_CORE:(bh + 1) * B_CORE].reshape(128, 4 * B_CORE)
    ).astype(np.float32) for bh in range(N_BH)]
    wt = np.ascontiguousarray(
        W_w.T.reshape(2, 128, BETA).transpose(1, 0, 2).reshape(128, 2 * BETA)
    ).astype(BF16)
    bw = beta_w[0].astype(np.float32)
    bwr = np.ascontiguousarray(
        np.repeat(bw.reshape(4, 128).T[:, :, None], 128, axis=2).reshape(128, BETA)
    ).astype(BF16)
    return qs_h, wt, bwr


_PROFILE = False
_LAST_PERF = {}


def kernel(**inputs):
    from concourse.bass_utils import run_bass_kernel_spmd

    V = np.asarray(inputs["V"], dtype=np.float32)
    h_t = np.asarray(inputs["h_t"], dtype=np.float32)
    W_h_w = np.asarray(inputs["W_h_w"], dtype=np.float32)
    W_h_b = np.asarray(inputs["W_h_b"], dtype=np.float32)
    W_w = np.asarray(inputs["W_w"], dtype=np.float32)
    W_b = np.asarray(inputs["W_b"], dtype=np.float32)
    beta_w = np.asarray(inputs["beta_w"], dtype=np.float32)
    beta_b = np.asarray(inputs["beta_b"], dtype=np.float32)

    qs_h, wt, bwr = _host_smalls(h_t, W_h_w, W_h_b, W_w, W_b, beta_w)

    rows = HP // N_HPQ
    Vb = V.astype(BF16)
    in_maps = []
    core_meta = []
    for k in range(N_HPQ):
        Vq = Vb[k * rows:(k + 1) * rows].reshape(S_CORE, C_DIM, B)
        for bh in range(N_BH):
            # [s, c, b-half] -> [c, b, s] contiguous
            vk = np.ascontiguousarray(
                Vq[:, :, bh * B_CORE:(bh + 1) * B_CORE].transpose(1, 2, 0))
            in_maps.append({"v": vk, "wt": wt, "qs": qs_h[bh], "bwr": bwr})
            core_meta.append(bh)

    nc = _get_nc()
    res = run_bass_kernel_spmd(nc, in_maps, core_ids=list(range(NCORES)),
                               trace=_PROFILE)
    if _PROFILE:
        _LAST_PERF["exec_time_ns"] = res.exec_time_ns
        _LAST_PERF["trace"] = res.instructions_and_trace
    P = np.zeros((C_DIM, B), np.float64)
    SE = np.zeros((B,), np.float64)
    for bh, r in zip(core_meta, res.results):
        sl = slice(bh * B_CORE, (bh + 1) * B_CORE)
        P[:, sl] += r["p_out"].reshape(C_DIM, B_CORE)
        SE[sl] += r["se_out"][0]
    # softmax is shift-invariant so beta_b cancels; no max-sub needed (|E|<=~8)
    C = (P / SE).T.reshape(B, 1, C_DIM)
    return C.astype(np.float32)
